# revision 44
# baseline (speedup 1.0000x reference)
"""CohortAwareBlock Trainium2 kernel.

Data-parallel over batch B=8 across 8 NeuronCores (one sample per core).
Cohort routing (gather of cohort_q_w by per-sample cohort id) happens on the
host while building each core's weight tensors; the device kernel is a plain
attention block.

Numerics: fp16 matmul inputs everywhere (same PE rate as bf16, ~8x less
noise); optionally the QK projection runs as fp8-e4m3 DoubleRow matmuls
(QK_FP8) with weights pre-scaled x32 to dodge fp8 subnormals and the inverse
scale folded into the exp's `scale` argument.

Per-core structure:
  qk^T [2048, N]  (QK-gen; fp8-DR or fp16)
  v_aug [keys, h, 65] fp16 (V-gen; col 64 = 1.0 so the flipped AV emits the
                            softmax denominator per q-partition)
  per (q-quarter, head pair):
    scores -> 2-bank PSUM [128, 4x256] -> ACT exp (fp16, batched) ->
    flipped attn@v: out [128 q, 65] per (head, q-128-chunk); col 64 = den ->
    DVE reciprocal [128,1] + tensor_scalar_mul -> nm_qd fp16 [q, d] layout
  per q-quarter: DMA-XBAR transpose nm_qd -> nmT [d, q] -> proj (fp16) + bias

PE emission is software-pipelined: scores of iteration i+1 are queued before
attn@v of iteration i so the in-order PE never waits on ACT's exp; QK/V
generation and the projection fill PE slack under the ACT-bound exp window.
"""

import numpy as np

import concourse.bass as bass
import concourse.bacc as bacc
import concourse.mybir as mybir
import concourse.tile as tile
from concourse.bass_utils import run_bass_kernel_spmd

P = 128
N = 1024            # sequence length
D = 1024            # model dim
H = 16              # heads
HD = 64             # head dim
NQ = 4              # q-quarters (256 q each)
QW = N // NQ        # 256
SCALE = HD ** -0.5
NCORES = 8

QK_FP8 = True       # fp8-e4m3 DoubleRow QK-gen (err ~1.4e-2) vs fp16 (~4e-4)
WS = 32.0 if QK_FP8 else 1.0
EXP_SCALE = SCALE / (WS * WS)

F32 = mybir.dt.float32
FP16 = mybir.dt.float16
FP8 = mybir.dt.float8e4
DR = mybir.MatmulPerfMode.DoubleRow
EXP = mybir.ActivationFunctionType.Exp


def build_nc():
    nc = bacc.Bacc(
        "TRN2",
        target_bir_lowering=False,
        debug=False,
        num_devices=NCORES,
    )

    # ---- external I/O (per-core shards, host-prepped layouts) ----
    xt = nc.dram_tensor("xt", [P, 8, N], FP16, kind="ExternalInput")   # x^T
    if QK_FP8:
        # DoubleRow-interleaved d-dim: d = (t2*2 + j)*128 + p
        xdr = nc.dram_tensor("xdr", [P, 4, 2, N], FP8, kind="ExternalInput")
        wqk = nc.dram_tensor("wqk", [P, 16, 4, 2, P], FP8, kind="ExternalInput")
    else:
        xdr = None
        wqk = nc.dram_tensor("wqk", [P, 16, 8, P], FP16, kind="ExternalInput")
    bqk = nc.dram_tensor("bqk", [P, 16], F32, kind="ExternalInput")
    wv = nc.dram_tensor("wv", [P, 8, D], FP16, kind="ExternalInput")
    bv = nc.dram_tensor("bv", [D], F32, kind="ExternalInput")
    wp = nc.dram_tensor("wp", [P, 8, D], FP16, kind="ExternalInput")
    bp = nc.dram_tensor("bp", [D], F32, kind="ExternalInput")
    out = nc.dram_tensor("out", [N, D], F32, kind="ExternalOutput")

    with tile.TileContext(nc) as tc:
        kernel_body(tc, xt, xdr, wqk, bqk, wv, bv, wp, bp, out)
    nc.compile()
    return nc


def kernel_body(tc, xt, xdr, wqk, bqk, wv, bv, wp, bp, out):
    nc = tc.nc
    from contextlib import ExitStack

    with ExitStack() as ctx:
        ctx.enter_context(
            nc.allow_low_precision(reason="fp16/fp8 matmul inputs by design")
        )
        res = ctx.enter_context(tc.tile_pool(name="res", bufs=1))
        gen_ps = ctx.enter_context(tc.tile_pool(name="gen_ps", bufs=2, space="PSUM"))
        sc_ps = ctx.enter_context(tc.tile_pool(name="sc_ps", bufs=2, space="PSUM"))
        av_ps = ctx.enter_context(tc.tile_pool(name="av_ps", bufs=2, space="PSUM"))
        exp_pool = ctx.enter_context(tc.tile_pool(name="exp_pool", bufs=28))
        rc_pool = ctx.enter_context(tc.tile_pool(name="rc_pool", bufs=4))
        avt_pool = ctx.enter_context(tc.tile_pool(name="avt_pool", bufs=6))
        nm_pool = ctx.enter_context(tc.tile_pool(name="nm_pool", bufs=3))
        oev_pool = ctx.enter_context(tc.tile_pool(name="oev_pool", bufs=2))

        # ---- resident tiles ----
        bqk_sb = res.tile([P, 16], F32)

        if QK_FP8:
            xdr_sb = res.tile([P, 4, 2, N], FP8)
            for t2 in range(4):
                nc.sync.dma_start(xdr_sb[:, t2], xdr[:, t2])
            wqk_sb = res.tile([P, 16, 4, 2, P], FP8)
        else:
            wqk_sb = res.tile([P, 16, 8, P], FP16)
        for co in range(8):
            nc.sync.dma_start(wqk_sb[:, co], wqk[:, co])
            nc.sync.dma_start(wqk_sb[:, 8 + co], wqk[:, 8 + co])
            if co == 0:
                nc.sync.dma_start(bqk_sb[:], bqk[:])

        # wv/x^T follow the wqk chunks on the sync queue; wp/bp are issued
        # later on the ACT queue (behind the first exps) since the projection
        # needs them only ~40us in
        wv_sb = res.tile([P, 8, D], FP16)
        xt_sb = res.tile([P, 8, N], FP16)
        bv_rep = res.tile([P, D], F32)
        nc.sync.dma_start(bv_rep[:], bv[None, :].to_broadcast([P, D]))
        for dc in range(8):
            nc.sync.dma_start(wv_sb[:, dc], wv[:, dc])
        for dc in range(8):
            nc.sync.dma_start(xt_sb[:, dc], xt[:, dc])
        wp_sb = res.tile([P, 8, D], FP16)
        bp_rep = res.tile([P, D], F32)
        for co in range(8):
            nc.sync.dma_start(wp_sb[:, co], wp[:, co])
        nc.sync.dma_start(bp_rep[:], bp[None, :].to_broadcast([P, D]))

        # v_aug[p, kt, h, :]: cols 0:64 = v for head h at key chunk kt,
        # col 64 = 1.0 (flipped attn@v then emits the softmax denominator
        # in output column 64, one value per q-partition)
        v_aug = res.tile([P, 8, H, HD + 1], FP16)
        nc.gpsimd.memset(v_aug[:, :, :, HD : HD + 1], 1.0)

        qk_sb = res.tile([P, 16, N], FP16)      # co 0..7 = q chunks, 8..15 = k
        # transposed normalized att, packed for proj: [d-part, qc, co, q]
        nmT = res.tile([P, 8, 8, P], FP16)

        # ---------------- emission helpers ----------------
        def qk_nh(co, nh):
            # half of a qk chunk: one psum group + evac (on Pool)
            ps = gen_ps.tile([P, 512], F32, tag="gps")
            if QK_FP8:
                for t2 in range(4):
                    nc.tensor.matmul(
                        ps[:],
                        lhsT=wqk_sb[:, co, t2],
                        rhs=xdr_sb[:, t2, :, nh * 512 : (nh + 1) * 512],
                        start=(t2 == 0),
                        stop=(t2 == 3),
                        perf_mode=DR,
                    )
            else:
                for dc in range(8):
                    nc.tensor.matmul(
                        ps[:],
                        lhsT=wqk_sb[:, co, dc],
                        rhs=xt_sb[:, dc, nh * 512 : (nh + 1) * 512],
                        start=(dc == 0),
                        stop=(dc == 7),
                    )
            nc.vector.tensor_scalar_add(
                qk_sb[:, co, nh * 512 : (nh + 1) * 512],
                ps[:],
                bqk_sb[:, co : co + 1],
            )

        def v_halves(eh, nt):
            # v[keys nt-chunk, 512 cols of eh] split into two PE units
            # sharing one psum accumulation group
            hold = []

            def a():
                ps = gen_ps.tile([P, 512], F32, tag="gps")
                hold.append(ps)
                for dc in range(4):
                    nc.tensor.matmul(
                        ps[:],
                        lhsT=xt_sb[:, dc, nt * P : (nt + 1) * P],
                        rhs=wv_sb[:, dc, eh * 512 : (eh + 1) * 512],
                        start=(dc == 0),
                        stop=False,
                    )

            def b():
                ps = hold[0]
                for dc in range(4, 8):
                    nc.tensor.matmul(
                        ps[:],
                        lhsT=xt_sb[:, dc, nt * P : (nt + 1) * P],
                        rhs=wv_sb[:, dc, eh * 512 : (eh + 1) * 512],
                        start=False,
                        stop=(dc == 7),
                    )
                nc.vector.tensor_add(
                    v_aug[:, nt, eh * 8 : (eh + 1) * 8, 0:HD],
                    ps[:].rearrange("p (h d) -> p h d", d=HD),
                    bv_rep[:, eh * 512 : (eh + 1) * 512].rearrange(
                        "p (h d) -> p h d", d=HD
                    ),
                )

            return a, b

        def sc_group(qh, co, g, hh):
            # one kt-group of scores + its batched exp; returns the exp tile
            q0 = qh * QW
            b0 = hh * HD
            ps = sc_ps.tile([P, 4 * QW], F32, tag="scps")
            for ki in range(4):
                kt = g * 4 + ki
                nc.tensor.matmul(
                    ps[:, ki * QW : (ki + 1) * QW],
                    lhsT=qk_sb[b0 : b0 + HD, 8 + co, kt * P : (kt + 1) * P],
                    rhs=qk_sb[b0 : b0 + HD, co, q0 : q0 + QW],
                    start=True,
                    stop=True,
                )
            ex = exp_pool.tile([P, 4, QW], FP16, tag="exp")
            nc.scalar.activation(
                ex[:], ps[:].rearrange("p (k q) -> p k q", k=4),
                EXP, scale=EXP_SCALE,
            )
            return ex

        def av_halves(qh, co, exps):
            # flipped attn@v for one head pair, split per head; the second
            # half also emits the batched XBAR transpose into nmT
            hold = []

            def half(hh):
                h = 2 * co + hh
                if hh == 0:
                    hold.append(nm_pool.tile([P, 2, 2, HD], FP16, tag="nm", name="nm"))
                nm = hold[0]
                for qs in range(2):
                    ps = av_ps.tile([P, HD + 1], F32, tag="avps")
                    for kt in range(8):
                        nc.tensor.matmul(
                            ps[:],
                            lhsT=exps[(hh, kt // 4)][:, kt % 4,
                                                     qs * P : (qs + 1) * P],
                            rhs=v_aug[:, kt, h, :],
                            start=(kt == 0),
                            stop=(kt == 7),
                        )
                    rc = rc_pool.tile([P, 1], F32, tag="rc", name="rc")
                    nc.vector.reciprocal(rc[:], ps[:, HD : HD + 1])
                    nc.vector.tensor_scalar_mul(
                        nm[:, qs, hh, :], ps[:, 0:HD], rc[:]
                    )
                if hh == 1:
                    nc.sync.dma_start(
                        nmT[:, qh * 2 : qh * 2 + 2, co, :],
                        nm[:].rearrange("p a b d -> p (a b d)"),
                        transpose=True,
                    )

            return (lambda: half(0)), (lambda: half(1))

        def pj_halves(qh, nt, fh):
            # one projection output group split into two PE units
            qc = qh * 2 + nt
            n0 = qc * P
            hold = []

            def a():
                ps = gen_ps.tile([P, 512], F32, tag="gps")
                hold.append(ps)
                for co in range(4):
                    nc.tensor.matmul(
                        ps[:],
                        lhsT=nmT[:, qc, co, :],
                        rhs=wp_sb[:, co, fh * 512 : (fh + 1) * 512],
                        start=(co == 0),
                        stop=False,
                    )

            def b():
                ps = hold[0]
                for co in range(4, 8):
                    nc.tensor.matmul(
                        ps[:],
                        lhsT=nmT[:, qc, co, :],
                        rhs=wp_sb[:, co, fh * 512 : (fh + 1) * 512],
                        start=False,
                        stop=(co == 7),
                    )
                ev = oev_pool.tile([P, 512], F32, tag="oev")
                nc.vector.tensor_add(
                    ev[:], ps[:], bp_rep[:, fh * 512 : (fh + 1) * 512]
                )
                nc.scalar.dma_start(
                    out[n0 : n0 + P, fh * 512 : (fh + 1) * 512], ev[:]
                )

            return a, b

        # ---------------- schedule ----------------
        # Greedy merge with virtual clocks: pe_t/act_t track each engine's
        # busy-until time under the cost model (score group 428ns PE +
        # 1038ns ACT; filler units carry their PE cost). Filler is emitted
        # while ACT has >1.3us of backlog; otherwise the next score group
        # goes out. sc_ps double-buffering caps how far PE runs ahead.
        from collections import deque

        qk_nh(0, 0)
        qk_nh(0, 1)
        qk_nh(8, 0)
        qk_nh(8, 1)

        v_units = [(eh, nt) for eh in range(2) for nt in range(8)]
        vi = 0
        av_q = deque()
        proj_q = deque()
        fq = deque()            # (cost_ns, thunk, exp_tiles_freed)
        pe_t = 4400.0           # first score matmul ~ after xdr+wqk DMA
        act_t = 0.0
        exp_starts = []

        def av_ready(nvi):
            if not av_q:
                return False
            qh0, co0, _ = av_q[0]
            need = 8 if (qh0 == 0 and co0 < 4) else 16
            return nvi >= need

        for qh in range(NQ):
            for co in range(8):
                it = qh * 8 + co
                if qh == 0 and co >= 1:
                    for nh in range(2):
                        fq.append((428, (lambda c, n: lambda: qk_nh(c, n))(co, nh), 0))
                        fq.append(
                            (428, (lambda c, n: lambda: qk_nh(8 + c, n))(co, nh), 0)
                        )
                if pe_t > 18000 and vi < 16:
                    for _ in range(2):
                        if vi < 16:
                            a, b = v_halves(*v_units[vi])
                            fq.append((852, a, 0))
                            fq.append((852, b, 0))
                            vi += 1
                keep = 4 if it < 28 else 1
                n = 0
                while len(av_q) > keep and n < 3 and av_ready(vi):
                    item = av_q.popleft()
                    a, b = av_halves(*item)
                    fq.append((440, a, 0))
                    fq.append((440, b, 4))
                    n += 1
                    if item[1] == 7:
                        for nt in range(2):
                            for fh in range(2):
                                proj_q.append((item[0], nt, fh))
                if proj_q:
                    a, b = pj_halves(*proj_q.popleft())
                    fq.append((852, a, 0))
                    fq.append((852, b, 0))
                # exp-pool liveness guard: queued-but-unexecuted AV halves
                # keep exp tiles alive; force-drain before allocating 4 more
                av_fq = sum(e[2] for e in fq)
                while fq and 4 * len(av_q) + av_fq + 8 > 28:
                    c, t, fr = fq.popleft()
                    t()
                    pe_t += c
                    av_fq -= fr
                exps = {}
                for g in range(2):
                    for hh in range(2):
                        ni = len(exp_starts)
                        cap = exp_starts[ni - 2] + 1038 if ni >= 2 else 0.0
                        while pe_t < cap - 50 or (act_t - pe_t) > 1300:
                            if not fq:
                                if proj_q:
                                    pa, pb = pj_halves(*proj_q.popleft())
                                    fq.append((852, pa, 0))
                                    fq.append((852, pb, 0))
                                else:
                                    break
                            c, t, _ = fq.popleft()
                            t()
                            pe_t += c
                        pe_t = max(pe_t, cap) + 428
                        st = max(act_t, pe_t)
                        exp_starts.append(st)
                        act_t = st + 1038
                        exps[(hh, g)] = sc_group(qh, co, g, hh)
                av_q.append((qh, co, exps))
        while av_q:
            a, b = av_halves(*av_q.popleft())
            a()
            b()
        while fq:
            fq.popleft()[1]()
        proj_q.extend((NQ - 1, nt, fh) for nt in range(2) for fh in range(2))
        while proj_q:
            a, b = pj_halves(*proj_q.popleft())
            a()
            b()





# revision 48
# speedup vs baseline: 1.0027x; 1.0027x over previous
"""CohortAwareBlock Trainium2 kernel.

Data-parallel over batch B=8 across 8 NeuronCores (one sample per core).
Cohort routing (gather of cohort_q_w by per-sample cohort id) happens on the
host while building each core's weight tensors; the device kernel is a plain
attention block.

Numerics: fp16 matmul inputs everywhere (same PE rate as bf16, ~8x less
noise); optionally the QK projection runs as fp8-e4m3 DoubleRow matmuls
(QK_FP8) with weights pre-scaled x32 to dodge fp8 subnormals and the inverse
scale folded into the exp's `scale` argument.

Per-core structure:
  qk^T [2048, N]  (QK-gen; fp8-DR or fp16)
  v_aug [keys, h, 65] fp16 (V-gen; col 64 = 1.0 so the flipped AV emits the
                            softmax denominator per q-partition)
  per (q-quarter, head pair):
    scores -> 2-bank PSUM [128, 4x256] -> ACT exp (fp16, batched) ->
    flipped attn@v: out [128 q, 65] per (head, q-128-chunk); col 64 = den ->
    DVE reciprocal [128,1] + tensor_scalar_mul -> nm_qd fp16 [q, d] layout
  per q-quarter: DMA-XBAR transpose nm_qd -> nmT [d, q] -> proj (fp16) + bias

PE emission is software-pipelined: scores of iteration i+1 are queued before
attn@v of iteration i so the in-order PE never waits on ACT's exp; QK/V
generation and the projection fill PE slack under the ACT-bound exp window.
"""

import numpy as np

import concourse.bass as bass
import concourse.bacc as bacc
import concourse.mybir as mybir
import concourse.tile as tile
from concourse.bass_utils import run_bass_kernel_spmd

P = 128
N = 1024            # sequence length
D = 1024            # model dim
H = 16              # heads
HD = 64             # head dim
NQ = 4              # q-quarters (256 q each)
QW = N // NQ        # 256
SCALE = HD ** -0.5
NCORES = 8

QK_FP8 = True       # fp8-e4m3 DoubleRow QK-gen (err ~1.4e-2) vs fp16 (~4e-4)
WS = 32.0 if QK_FP8 else 1.0
EXP_SCALE = SCALE / (WS * WS)

F32 = mybir.dt.float32
FP16 = mybir.dt.float16
BF16 = mybir.dt.bfloat16
FP8 = mybir.dt.float8e4
DR = mybir.MatmulPerfMode.DoubleRow
EXP = mybir.ActivationFunctionType.Exp


def build_nc():
    nc = bacc.Bacc(
        "TRN2",
        target_bir_lowering=False,
        debug=False,
        num_devices=NCORES,
    )

    # ---- external I/O (per-core shards, host-prepped layouts) ----
    xt = nc.dram_tensor("xt", [P, 8, N], FP16, kind="ExternalInput")   # x^T
    if QK_FP8:
        # DoubleRow-interleaved d-dim: d = (t2*2 + j)*128 + p
        xdr = nc.dram_tensor("xdr", [P, 4, 2, N], FP8, kind="ExternalInput")
        wqk = nc.dram_tensor("wqk", [P, 16, 4, 2, P], FP8, kind="ExternalInput")
    else:
        xdr = None
        wqk = nc.dram_tensor("wqk", [P, 16, 8, P], FP16, kind="ExternalInput")
    bqk = nc.dram_tensor("bqk", [P, 16], F32, kind="ExternalInput")
    wv = nc.dram_tensor("wv", [P, 8, D], FP16, kind="ExternalInput")
    bv = nc.dram_tensor("bv", [D], BF16, kind="ExternalInput")
    wp = nc.dram_tensor("wp", [P, 8, D], FP16, kind="ExternalInput")
    bp = nc.dram_tensor("bp", [D], BF16, kind="ExternalInput")
    out = nc.dram_tensor("out", [N, D], F32, kind="ExternalOutput")

    with tile.TileContext(nc) as tc:
        kernel_body(tc, xt, xdr, wqk, bqk, wv, bv, wp, bp, out)
    nc.compile()
    return nc


def kernel_body(tc, xt, xdr, wqk, bqk, wv, bv, wp, bp, out):
    nc = tc.nc
    from contextlib import ExitStack

    with ExitStack() as ctx:
        ctx.enter_context(
            nc.allow_low_precision(reason="fp16/fp8 matmul inputs by design")
        )
        res = ctx.enter_context(tc.tile_pool(name="res", bufs=1))
        gen_ps = ctx.enter_context(tc.tile_pool(name="gen_ps", bufs=2, space="PSUM"))
        sc_ps = ctx.enter_context(tc.tile_pool(name="sc_ps", bufs=2, space="PSUM"))
        av_ps = ctx.enter_context(tc.tile_pool(name="av_ps", bufs=2, space="PSUM"))
        exp_pool = ctx.enter_context(tc.tile_pool(name="exp_pool", bufs=30))
        rc_pool = ctx.enter_context(tc.tile_pool(name="rc_pool", bufs=4))
        nm_pool = ctx.enter_context(tc.tile_pool(name="nm_pool", bufs=3))
        oev_pool = ctx.enter_context(tc.tile_pool(name="oev_pool", bufs=2))

        # ---- resident tiles ----
        bqk_sb = res.tile([P, 16], F32)

        if QK_FP8:
            xdr_sb = res.tile([P, 4, 2, N], FP8)
            for t2 in range(4):
                nc.sync.dma_start(xdr_sb[:, t2], xdr[:, t2])
            wqk_sb = res.tile([P, 16, 4, 2, P], FP8)
        else:
            wqk_sb = res.tile([P, 16, 8, P], FP16)
        for co in range(8):
            nc.sync.dma_start(wqk_sb[:, co], wqk[:, co])
            nc.sync.dma_start(wqk_sb[:, 8 + co], wqk[:, 8 + co])
            if co == 0:
                nc.sync.dma_start(bqk_sb[:], bqk[:])

        # wv/x^T/wp/bp follow the wqk chunks on the sync queue in need-order
        # (V-gen ~20us in, projection ~45us in); out-DMAs use the ACT queue
        wv_sb = res.tile([P, 8, D], FP16)
        xt_sb = res.tile([P, 8, N], FP16)
        bv_rep = res.tile([P, D], BF16)
        nc.sync.dma_start(bv_rep[:], bv[None, :].to_broadcast([P, D]))
        for dc in range(8):
            nc.sync.dma_start(wv_sb[:, dc], wv[:, dc])
        for dc in range(8):
            nc.sync.dma_start(xt_sb[:, dc], xt[:, dc])
        wp_sb = res.tile([P, 8, D], FP16)
        bp_rep = res.tile([P, D], BF16)
        for co in range(8):
            nc.sync.dma_start(wp_sb[:, co], wp[:, co])
        nc.sync.dma_start(bp_rep[:], bp[None, :].to_broadcast([P, D]))

        # v_aug[p, kt, h, :]: cols 0:64 = v for head h at key chunk kt,
        # col 64 = 1.0 (flipped attn@v then emits the softmax denominator
        # in output column 64, one value per q-partition)
        v_aug = res.tile([P, 8, H, HD + 1], FP16)
        nc.gpsimd.memset(v_aug[:, :, :, HD : HD + 1], 1.0)

        qk_sb = res.tile([P, 16, N], FP16)      # co 0..7 = q chunks, 8..15 = k
        # transposed normalized att, packed for proj: [d-part, qc, co, q]
        nmT = res.tile([P, 8, 8, P], FP16)

        # ---------------- emission helpers ----------------
        def qk_nh(co, nh):
            # half of a qk chunk: one psum group + bias evac (GPSIMD cannot
            # read PSUM on real hardware, so evacs run on DVE)
            ps = gen_ps.tile([P, 512], F32, tag="gps")
            if QK_FP8:
                for t2 in range(4):
                    nc.tensor.matmul(
                        ps[:],
                        lhsT=wqk_sb[:, co, t2],
                        rhs=xdr_sb[:, t2, :, nh * 512 : (nh + 1) * 512],
                        start=(t2 == 0),
                        stop=(t2 == 3),
                        perf_mode=DR,
                    )
            else:
                for dc in range(8):
                    nc.tensor.matmul(
                        ps[:],
                        lhsT=wqk_sb[:, co, dc],
                        rhs=xt_sb[:, dc, nh * 512 : (nh + 1) * 512],
                        start=(dc == 0),
                        stop=(dc == 7),
                    )
            nc.vector.tensor_scalar_add(
                qk_sb[:, co, nh * 512 : (nh + 1) * 512],
                ps[:],
                bqk_sb[:, co : co + 1],
            )

        def v_halves(eh, nt):
            # v[keys nt-chunk, 512 cols of eh] split into two PE units
            # sharing one psum accumulation group
            hold = []

            def a():
                ps = gen_ps.tile([P, 512], F32, tag="gps")
                hold.append(ps)
                for dc in range(4):
                    nc.tensor.matmul(
                        ps[:],
                        lhsT=xt_sb[:, dc, nt * P : (nt + 1) * P],
                        rhs=wv_sb[:, dc, eh * 512 : (eh + 1) * 512],
                        start=(dc == 0),
                        stop=False,
                    )

            def b():
                ps = hold[0]
                for dc in range(4, 8):
                    nc.tensor.matmul(
                        ps[:],
                        lhsT=xt_sb[:, dc, nt * P : (nt + 1) * P],
                        rhs=wv_sb[:, dc, eh * 512 : (eh + 1) * 512],
                        start=False,
                        stop=(dc == 7),
                    )
                nc.vector.tensor_add(
                    v_aug[:, nt, eh * 8 : (eh + 1) * 8, 0:HD],
                    ps[:].rearrange("p (h d) -> p h d", d=HD),
                    bv_rep[:, eh * 512 : (eh + 1) * 512].rearrange(
                        "p (h d) -> p h d", d=HD
                    ),
                )

            return a, b

        def sc_group(qh, co, g, hh):
            # one kt-group of scores + its batched exp; returns the exp tile
            q0 = qh * QW
            b0 = hh * HD
            ps = sc_ps.tile([P, 4 * QW], F32, tag="scps")
            for ki in range(4):
                kt = g * 4 + ki
                nc.tensor.matmul(
                    ps[:, ki * QW : (ki + 1) * QW],
                    lhsT=qk_sb[b0 : b0 + HD, 8 + co, kt * P : (kt + 1) * P],
                    rhs=qk_sb[b0 : b0 + HD, co, q0 : q0 + QW],
                    start=True,
                    stop=True,
                )
            ex = exp_pool.tile([P, 4, QW], FP16, tag="exp")
            nc.scalar.activation(
                ex[:], ps[:].rearrange("p (k q) -> p k q", k=4),
                EXP, scale=EXP_SCALE,
            )
            return ex

        def av_halves(qh, co, exps):
            # flipped attn@v for one head pair, split per head; the second
            # half also emits the batched XBAR transpose into nmT
            hold = []

            def half(hh):
                h = 2 * co + hh
                if hh == 0:
                    hold.append(nm_pool.tile([P, 2, 2, HD], FP16, tag="nm", name="nm"))
                nm = hold[0]
                for qs in range(2):
                    ps = av_ps.tile([P, HD + 1], F32, tag="avps")
                    for kt in range(8):
                        nc.tensor.matmul(
                            ps[:],
                            lhsT=exps[(hh, kt // 4)][:, kt % 4,
                                                     qs * P : (qs + 1) * P],
                            rhs=v_aug[:, kt, h, :],
                            start=(kt == 0),
                            stop=(kt == 7),
                        )
                    rc = rc_pool.tile([P, 1], F32, tag="rc", name="rc")
                    nc.vector.reciprocal(rc[:], ps[:, HD : HD + 1])
                    nc.vector.tensor_scalar_mul(
                        nm[:, qs, hh, :], ps[:, 0:HD], rc[:]
                    )
                if hh == 1:
                    nc.sync.dma_start(
                        nmT[:, qh * 2 : qh * 2 + 2, co, :],
                        nm[:].rearrange("p a b d -> p (a b d)"),
                        transpose=True,
                    )

            return (lambda: half(0)), (lambda: half(1))

        def pj_halves(qh, nt, fh):
            # one projection output group split into two PE units
            qc = qh * 2 + nt
            n0 = qc * P
            hold = []

            def a():
                ps = gen_ps.tile([P, 512], F32, tag="gps")
                hold.append(ps)
                for co in range(4):
                    nc.tensor.matmul(
                        ps[:],
                        lhsT=nmT[:, qc, co, :],
                        rhs=wp_sb[:, co, fh * 512 : (fh + 1) * 512],
                        start=(co == 0),
                        stop=False,
                    )

            def b():
                ps = hold[0]
                for co in range(4, 8):
                    nc.tensor.matmul(
                        ps[:],
                        lhsT=nmT[:, qc, co, :],
                        rhs=wp_sb[:, co, fh * 512 : (fh + 1) * 512],
                        start=False,
                        stop=(co == 7),
                    )
                ev = oev_pool.tile([P, 512], F32, tag="oev")
                nc.vector.tensor_add(
                    ev[:], ps[:], bp_rep[:, fh * 512 : (fh + 1) * 512]
                )
                nc.scalar.dma_start(
                    out[n0 : n0 + P, fh * 512 : (fh + 1) * 512], ev[:]
                )

            return a, b

        # ---------------- schedule ----------------
        # Greedy merge with virtual clocks: pe_t/act_t track each engine's
        # busy-until time under the cost model (score group 428ns PE +
        # 1038ns ACT; filler units carry their PE cost). Filler is emitted
        # while ACT has >1.3us of backlog; otherwise the next score group
        # goes out. sc_ps double-buffering caps how far PE runs ahead.
        from collections import deque

        qk_nh(0, 0)
        qk_nh(0, 1)
        qk_nh(8, 0)
        qk_nh(8, 1)

        v_units = [(eh, nt) for eh in range(2) for nt in range(8)]
        vi = 0
        av_q = deque()
        proj_q = deque()
        fq = deque()            # (cost_ns, thunk, exp_tiles_freed)
        pe_t = 4400.0           # first score matmul ~ after xdr+wqk DMA
        act_t = 0.0
        exp_starts = []

        def av_ready(nvi):
            if not av_q:
                return False
            qh0, co0, _ = av_q[0]
            need = 8 if (qh0 == 0 and co0 < 4) else 16
            return nvi >= need

        for qh in range(NQ):
            for co in range(8):
                it = qh * 8 + co
                if qh == 0 and co >= 1:
                    for nh in range(2):
                        fq.append((428, (lambda c, n: lambda: qk_nh(c, n))(co, nh), 0))
                        fq.append(
                            (428, (lambda c, n: lambda: qk_nh(8 + c, n))(co, nh), 0)
                        )
                if pe_t > 18000 and vi < 16:
                    for _ in range(2):
                        if vi < 16:
                            a, b = v_halves(*v_units[vi])
                            fq.append((852, a, 0))
                            fq.append((852, b, 0))
                            vi += 1
                keep = 4 if it < 28 else 1
                n = 0
                while len(av_q) > keep and n < 3 and av_ready(vi):
                    item = av_q.popleft()
                    a, b = av_halves(*item)
                    fq.append((440, a, 0))
                    fq.append((440, b, 4))
                    n += 1
                    if item[1] == 7:
                        for nt in range(2):
                            for fh in range(2):
                                proj_q.append((item[0], nt, fh))
                if proj_q:
                    a, b = pj_halves(*proj_q.popleft())
                    fq.append((852, a, 0))
                    fq.append((852, b, 0))
                # exp-pool liveness guard: queued-but-unexecuted AV halves
                # keep exp tiles alive; force-drain before allocating 4 more
                av_fq = sum(e[2] for e in fq)
                while fq and 4 * len(av_q) + av_fq + 8 > 30:
                    c, t, fr = fq.popleft()
                    t()
                    pe_t += c
                    av_fq -= fr
                exps = {}
                for g in range(2):
                    for hh in range(2):
                        ni = len(exp_starts)
                        cap = exp_starts[ni - 2] + 1038 if ni >= 2 else 0.0
                        while pe_t < cap - 50 or (act_t - pe_t) > 1300:
                            if not fq:
                                if proj_q:
                                    pa, pb = pj_halves(*proj_q.popleft())
                                    fq.append((852, pa, 0))
                                    fq.append((852, pb, 0))
                                else:
                                    break
                            c, t, _ = fq.popleft()
                            t()
                            pe_t += c
                        pe_t = max(pe_t, cap) + 428
                        st = max(act_t, pe_t)
                        exp_starts.append(st)
                        act_t = st + 1038
                        exps[(hh, g)] = sc_group(qh, co, g, hh)
                av_q.append((qh, co, exps))
        while av_q:
            a, b = av_halves(*av_q.popleft())
            a()
            b()
        while fq:
            fq.popleft()[1]()
        proj_q.extend((NQ - 1, nt, fh) for nt in range(2) for fh in range(2))
        while proj_q:
            a, b = pj_halves(*proj_q.popleft())
            a()
            b()



